# revision 6
# baseline (speedup 1.0000x reference)
"""AdaptiveGraphLearning forward on 8 Trainium2 NeuronCores.

Data-parallel over batch B=64: each core processes 8 batches; the (N,N)
adjacency parameter and tiny edge-MLP weights are replicated (the forward
pass needs no collectives).

Per-core dataflow (8 batches as 4 pairs):
  - HWDGE-DMA two batches of temporal_features per transfer as
    (128, 8192) f32 tiles: partition p=(b_lo,n), free=(h,t), split in
    halves that alternate between the SP and ACT HWDGE rings so both
    rings stream for the whole kernel (~430 GB/s combined).
  - Sum over t on DVE: one f32->bf16 fold, then bf16 folds in 2x mode
    (t:128->64->32->16) and a short reduce -> R (128=(b_lo,n), 128=h).
    TensorE transpose -> node_T bf16 (h on partitions); 1/T folded into
    the W1 halves host-side.
  - Edge MLP per batch b (all matmuls bf16, PSUM f32):
      PiT = node_b @ Wi, PjT = node_b @ Wj   (two 128-row matmuls;
        lhsT = node_T columns, rhs = Wi/Wj)  -> MT = [PiT; PjT] bf16
      per chunk c (8 i-rows x 64 j = 512 pairs):
        xp = MT.T @ S_c    one matmul against a constant 0/1 mask with
             ones at slots i and 64+j -> xp[h,(i,j)] = Pi[i,h]+Pj[j,h]
        x  = relu(xp + b1)                     ScalarE evacuation, bf16
        h2 = W2.T @ x -> PSUM partitions [64*(c%2), 64*(c%2)+64) so two
             chunks share one (128,512) PSUM tile
      per chunk pair d: h2_sb = relu(h2_ps + b2) on GpSimd (tensor_scalar
        (add b2) max 0), then F accum: w3_ps += W3pair_d.T @ h2_sb with
        one-hot block weights routing chunk 2d -> row 2d, 2d+1 -> 2d+1
        (0.25 symmetrization factor folded into W3).
  - F(8,512) -> F(64,64) via SBUF->SBUF DMA (identical linearized element
    order), F^T on TensorE, then
      out = (relu(G + F + F^T) + I) / (rowsum + 1e-8)
    with G = 0.25*(ap+ap^T) precomputed host-side; the relu/+I/rowsum are
    fused into one DVE scalar_tensor_tensor with accum_out.
  - Batch-b epilogue stages are interleaved into batch b+1's chunk loop so
    no engine FIFO head blocks on the SBUF->SBUF reinterpret DMA.

Harness notes: walrus in this container accepts a single semaphore wait
per instruction, so a BIR-level pass splits Tile's multi-wait
instructions onto EventSemaphore carriers; the Tile kernel-tail drain
gets the same treatment at build time.
"""
import sys

sys.path.insert(0, '/opt/trn_rl_repo')

import numpy as np

B, N, H, T = 64, 64, 128, 128
NCORES = 8
B_LOC = B // NCORES      # 8 batches per core
PAIRS = B_LOC // 2       # 4 batch pairs per core
NCH = N // 8             # 8 i-chunks per batch (8 i x 64 j = 512 wide)

_CACHE = {}


def _install_wait_splitter():
    """walrus's per-instruction sync structs hold a single semaphore wait;
    Tile can emit several on one instruction. Split extras onto preceding
    single-wait Drain instructions at the BIR-JSON level."""
    if _CACHE.get('wait_splitter'):
        return
    import json

    import concourse.bass2jax as bass2jax

    orig = bass2jax.compile_bir_kernel

    def split_waits_in_bir(bir_bytes):
        d = json.loads(bir_bytes)
        n_new = [0]
        for fn in d.get("functions", []):
            for bb in fn.get("blocks", []):
                out = []
                for ins in bb.get("instructions", []):
                    si = ins.get("sync_info") or {}
                    waits = si.get("on_wait") or []
                    if len(waits) > 1:
                        for w in waits[:-1]:
                            n_new[0] += 1
                            out.append({
                                "engine": ins["engine"],
                                "ins": [],
                                "name": f"IWS-{n_new[0]}",
                                "opcode": "EventSemaphore",
                                "outs": [],
                                "sync_info": {"on_update": [], "on_wait": [w]},
                            })
                        si["on_wait"] = [waits[-1]]
                    out.append(ins)
                bb["instructions"] = out
        return json.dumps(d).encode()

    def wrapper(ant_bir_str, *a, **kw):
        return orig(split_waits_in_bir(ant_bir_str), *a, **kw)

    bass2jax.compile_bir_kernel = wrapper
    _CACHE['wait_splitter'] = True


def _split_drain_tile_context(tile_mod, mybir, nc):
    """TileContext whose kernel-tail drain splits its semaphore waits across
    sequential Drain instructions (walrus CTRL insts accept one wait)."""
    from concourse.tile import ScopedClock

    class SplitDrainTileContext(tile_mod.TileContext):
        def _drain_and_barrier(self, tick_clock, wait_clock):
            drain_inst = self.nc.sync.drain()
            wait_clock.add_sem_waits(
                drain_inst.ins, ScopedClock({None: tick_clock.global_clock})
            )
            waits = list(drain_inst.ins.sync_info.on_wait)
            if len(waits) > 1:
                drain_inst.ins.sync_info = mybir.SyncInfo(
                    on_wait=waits[:1],
                    on_update=list(drain_inst.ins.sync_info.on_update),
                )
                for i in range(1, len(waits)):
                    extra = self.nc.sync.drain()
                    extra.ins.sync_info = mybir.SyncInfo(
                        on_wait=waits[i : i + 1], on_update=[]
                    )
            self.nc.all_engine_barrier()
            assert self.sems is not None
            popped = self.nc._tile_sem_poison_stack.pop()
            assert popped is self._sem_poison
            self.nc.clear_and_free_semaphores(list(self.sems.allocated().values()))
            self.nc.all_engine_barrier()

    return SplitDrainTileContext(nc)


def build_nc():
    import concourse.bass as bass
    import concourse.tile as tile
    from concourse import mybir
    from contextlib import ExitStack

    f32 = mybir.dt.float32
    bf16 = mybir.dt.bfloat16
    AF = mybir.ActivationFunctionType
    ALU = mybir.AluOpType
    AX = mybir.AxisListType

    nc = bass.Bass()
    tf = nc.declare_dram_parameter("tf", [B_LOC, N, H, T], f32, isOutput=False)
    Wi = nc.declare_dram_parameter("Wi", [H, H], bf16, isOutput=False)
    Wj = nc.declare_dram_parameter("Wj", [H, H], bf16, isOutput=False)
    b1c = nc.declare_dram_parameter("b1c", [H, 1], f32, isOutput=False)
    W2 = nc.declare_dram_parameter("W2", [H, H // 2], bf16, isOutput=False)
    b2p = nc.declare_dram_parameter("b2p", [H, 1], f32, isOutput=False)
    W3p = nc.declare_dram_parameter("W3p", [H, 8 * (NCH // 2)], bf16,
                                    isOutput=False)
    b3c = nc.declare_dram_parameter("b3c", [8, 1], f32, isOutput=False)
    Smask = nc.declare_dram_parameter("Smask", [H, NCH * 8 * N], bf16,
                                      isOutput=False)
    G = nc.declare_dram_parameter("G", [N, N], f32, isOutput=False)
    I64 = nc.declare_dram_parameter("I64", [N, N], f32, isOutput=False)
    I128 = nc.declare_dram_parameter("I128", [H, H], f32, isOutput=False)
    out_ext = nc.declare_dram_parameter("out", [B_LOC, N, N], f32, isOutput=True)

    NOBIAS = _CACHE.get('cfg_nobias', False)
    SMW = NCH * 8 * N  # 4096 mask columns

    with _split_drain_tile_context(tile, mybir, nc) as tc, ExitStack() as ctx:
        consts = ctx.enter_context(tc.tile_pool(name="consts", bufs=1))
        tf_pool = ctx.enter_context(tc.tile_pool(name="tf", bufs=4))
        fold_pool = ctx.enter_context(tc.tile_pool(name="fold", bufs=2))
        red_pool = ctx.enter_context(tc.tile_pool(name="red", bufs=2))
        mt_pool = ctx.enter_context(tc.tile_pool(name="mt", bufs=2))
        x_pool = ctx.enter_context(tc.tile_pool(name="x", bufs=4))
        h2_pool = ctx.enter_context(tc.tile_pool(name="h2", bufs=4))
        ff_pool = ctx.enter_context(tc.tile_pool(name="ff", bufs=2))
        ep_pool = ctx.enter_context(tc.tile_pool(name="ep", bufs=2))
        ps_xp = ctx.enter_context(tc.tile_pool(name="ps_xp", bufs=2, space="PSUM"))
        ps_h2 = ctx.enter_context(tc.tile_pool(name="ps_h2", bufs=2, space="PSUM"))
        ps_w3 = ctx.enter_context(tc.tile_pool(name="ps_w3", bufs=1, space="PSUM"))
        ps_pi = ctx.enter_context(tc.tile_pool(name="ps_pi", bufs=1, space="PSUM"))
        ps_t = ctx.enter_context(tc.tile_pool(name="ps_t", bufs=1, space="PSUM"))
        if True:
            # smask halves split across both rings so neither delays pair0
            sm_sb = consts.tile([H, SMW], bf16)
            nc.sync.dma_start(sm_sb[:, 0 : SMW // 2], Smask[:, 0 : SMW // 2])
            nc.scalar.dma_start(
                sm_sb[:, SMW // 2 : SMW], Smask[:, SMW // 2 : SMW])
            wi_sb = consts.tile([H, H], bf16)
            nc.scalar.dma_start(wi_sb[:], Wi[:])
            wj_sb = consts.tile([H, H], bf16)
            nc.scalar.dma_start(wj_sb[:], Wj[:])
            w2_sb = consts.tile([H, H // 2], bf16)
            nc.scalar.dma_start(w2_sb[:], W2[:])
            w3_sb = consts.tile([H, 8 * (NCH // 2)], bf16)
            nc.scalar.dma_start(w3_sb[:], W3p[:])
            b1_sb = consts.tile([H, 1], f32)
            nc.scalar.dma_start(b1_sb[:], b1c[:])
            b2_sb = consts.tile([H, 1], f32)
            nc.scalar.dma_start(b2_sb[:], b2p[:])
            b3_sb = consts.tile([8, 1], f32)
            nc.scalar.dma_start(b3_sb[:], b3c[:])
            g_sb = consts.tile([N, N], f32)
            nc.scalar.dma_start(g_sb[:], G[:])
            i64_sb = consts.tile([N, N], f32)
            nc.scalar.dma_start(i64_sb[:], I64[:])
            i128_sb = consts.tile([H, H], f32)
            nc.scalar.dma_start(i128_sb[:], I128[:])

            HQ = H // 2

            def load_pair(c):
                halves = []
                for hh in range(2):
                    tft = tf_pool.tile([128, HQ, T], f32, name=f"tft{c}_{hh}",
                                       tag="tft")
                    eng = nc.sync if hh == 0 else nc.scalar
                    eng.dma_start(
                        tft[:],
                        tf[2 * c : 2 * c + 2, :, hh * HQ : (hh + 1) * HQ, :])
                    halves.append(tft[:])
                return halves

            def load_pair0_quarters():
                # Pair 0 in 2MB quarters, h-low on SP / h-high on ACT ring:
                # the first fold starts right after the first quarter lands.
                qs = []
                HF = H // 4
                for q in range(4):
                    tft = tf_pool.tile([128, HF, T], f32, name=f"tfq{q}",
                                       tag="tft")
                    eng = nc.sync if q < 2 else nc.scalar
                    eng.dma_start(
                        tft[:], tf[0:2, :, q * HF : (q + 1) * HF, :])
                    qs.append(tft[:])
                return qs

            def emit_folds(c, parts):
                # Sum over T: R[p=(b_lo,n), h] = sum_t tf[2c+b_lo, n, h, t].
                r_sb = red_pool.tile([128, H], f32, tag="r", name=f"r{c}")
                npc = len(parts)
                HP = H // npc
                for hh in range(npc):
                    th = parts[hh]
                    f1 = fold_pool.tile([128, HP, 64], bf16, tag="f1",
                                        name=f"f1_{c}_{hh}")
                    nc.vector.tensor_tensor(
                        f1[:], th[:, :, 0:64], th[:, :, 64:128], op=ALU.add)
                    # bf16 fold levels ride the otherwise-idle GpSimd
                    f2 = fold_pool.tile([128, HP, 32], bf16, tag="f2",
                                        name=f"f2_{c}_{hh}")
                    nc.gpsimd.tensor_tensor(
                        f2[:], f1[:, :, 0:32], f1[:, :, 32:64], op=ALU.add)
                    f3 = fold_pool.tile([128, HP, 16], bf16, tag="f3",
                                        name=f"f3_{c}_{hh}")
                    nc.gpsimd.tensor_tensor(
                        f3[:], f2[:, :, 0:16], f2[:, :, 16:32], op=ALU.add)
                    nc.vector.reduce_sum(
                        r_sb[:, hh * HP : (hh + 1) * HP], f3[:], axis=AX.X)
                return r_sb

            def mlp_batch(b, rt_sb, tails):
                """Emit batch b's MLP; interleave `tails` (previous batch's
                epilogue stages) into the chunk loop. Returns this batch's
                tail stages."""
                b_lo = b % 2
                nodeb = rt_sb[:, 64 * b_lo : 64 * b_lo + 64]
                pi_ps = ps_pi.tile([128, H], f32, tag="pi", name=f"pi{b}")
                nc.tensor.matmul(pi_ps[0:64, :], nodeb, wi_sb[:],
                                 start=True, stop=True)
                nc.tensor.matmul(pi_ps[64:128, :], nodeb, wj_sb[:],
                                 start=True, stop=True)
                mt_sb = mt_pool.tile([128, H], bf16, tag="mt", name=f"mt{b}")
                nc.scalar.activation(mt_sb[:], pi_ps[:], AF.Copy)
                w3_ps = ps_w3.tile([8, 512], f32, tag="w3", name=f"w3_{b}")

                xs = {}
                h2ps = {}
                h2sb = {}
                ti = iter(tails)
                for c in range(NCH + 2):
                    if c < NCH:
                        xp = ps_xp.tile([128, 512], f32, tag="xp", name=f"xp{b}_{c}")
                        nc.tensor.matmul(
                            xp[:], mt_sb[:],
                            sm_sb[:, 512 * c : 512 * (c + 1)],
                            start=True, stop=True)
                        x_sb = x_pool.tile([128, 512], bf16, tag="x", name=f"x{b}_{c}")
                        nc.scalar.activation(
                            x_sb[:], xp[:], AF.Relu,
                            bias=0.0 if NOBIAS else b1_sb[:])
                        xs[c] = x_sb
                    k = c - 1
                    if 0 <= k < NCH:
                        d = k // 2
                        if k % 2 == 0:
                            h2ps[d] = ps_h2.tile([128, 512], f32, tag="h2ps", name=f"h2ps{b}_{d}")
                        nc.tensor.matmul(
                            h2ps[d][64 * (k % 2) : 64 * (k % 2) + 64, :],
                            w2_sb[:], xs[k][:], start=True, stop=True)
                        if k % 2 == 1:
                            # GpSimd can't read PSUM; split the relu(h2+b2)
                            # evacuations between ScalarE and DVE
                            hs = h2_pool.tile([128, 512], bf16, tag="h2", name=f"h2_{b}_{d}")
                            if d % 2 == 0:
                                nc.scalar.activation(
                                    hs[:], h2ps[d][:], AF.Relu,
                                    bias=0.0 if NOBIAS else b2_sb[:])
                            else:
                                nc.vector.tensor_scalar(
                                    hs[:], h2ps[d][:], scalar1=b2_sb[:],
                                    scalar2=0.0, op0=ALU.add, op1=ALU.max)
                            h2sb[d] = hs
                    k2 = c - 2
                    if 0 <= k2 < NCH and k2 % 2 == 1:
                        d = k2 // 2
                        nc.tensor.matmul(
                            w3_ps[:], w3_sb[:, 8 * d : 8 * d + 8],
                            h2sb[d][:],
                            start=(d == 0), stop=(d == NCH // 2 - 1))
                    if c in (1, 2, 4, 5, 6):
                        stage = next(ti, None)
                        if stage is not None:
                            stage()
                for stage in ti:
                    stage()

                # epilogue stages for this batch (emitted into b+1's loop)
                st = {}

                def s_ff():
                    ff_sb = ff_pool.tile([8, 512], f32, tag="ff", name=f"ff{b}")
                    if NOBIAS:
                        nc.scalar.activation(ff_sb[:], w3_ps[:], AF.Copy)
                    else:
                        nc.scalar.activation(ff_sb[:], w3_ps[:], AF.Identity,
                                             bias=b3_sb[:])
                    st['ff'] = ff_sb

                def s_fdma():
                    f_sb = ep_pool.tile([N, N], f32, tag="f", name=f"fsb{b}")
                    nc.sync.dma_start(f_sb[:], st['ff'][:])
                    st['f'] = f_sb

                def s_ft():
                    ft_ps = ps_t.tile([N, N], f32, tag="ft", name=f"ft{b}")
                    nc.tensor.transpose(ft_ps[:], st['f'][:], i64_sb[:, :64])
                    st['ft'] = ft_ps

                def s_epi():
                    f_sb, ft_ps = st['f'], st['ft']
                    t1 = ep_pool.tile([N, N], f32, tag="t1", name=f"t1_{b}")
                    nc.vector.tensor_tensor(t1[:], f_sb[:], ft_ps[:],
                                            op=ALU.add)
                    t2 = ep_pool.tile([N, N], f32, tag="t2", name=f"t2_{b}")
                    nc.vector.tensor_tensor(t2[:], t1[:], g_sb[:], op=ALU.add)
                    spi = ep_pool.tile([N, N], f32, tag="spi", name=f"spi{b}")
                    rs = ep_pool.tile([N, 1], f32, tag="rs", name=f"rs{b}")
                    nc.vector.scalar_tensor_tensor(
                        spi[:], t2[:], 0.0, i64_sb[:], op0=ALU.max,
                        op1=ALU.add, accum_out=rs[:])
                    rb = ep_pool.tile([N, 1], f32, tag="rb", name=f"rb{b}")
                    nc.vector.tensor_scalar(
                        rb[:], rs[:], scalar1=1e-8, scalar2=None, op0=ALU.add)
                    rec = ep_pool.tile([N, 1], f32, tag="rec", name=f"rec{b}")
                    nc.vector.reciprocal(rec[:], rb[:])
                    o_sb = ep_pool.tile([N, N], f32, tag="o", name=f"o{b}")
                    nc.vector.tensor_scalar(
                        o_sb[:], spi[:], scalar1=rec[:], scalar2=None,
                        op0=ALU.mult)
                    st['o'] = o_sb

                def s_out():
                    nc.sync.dma_start(out_ext[b], st['o'][:])

                return [s_ff, s_fdma, s_ft, s_epi, s_out]

            pending = load_pair0_quarters()
            tails = []
            for c in range(PAIRS):
                parts = pending
                if c + 1 < PAIRS:
                    pending = load_pair(c + 1)
                r_sb = emit_folds(c, parts)
                # node_T[h, (b_lo, n)] via TensorE transpose (f32 in, bf16 out)
                rt_ps = ps_t.tile([128, 128], f32, tag="rt", name=f"rt{c}")
                nc.tensor.transpose(rt_ps[:], r_sb[:], i128_sb[:])
                rt_sb = red_pool.tile([128, 128], bf16, tag="rt_sb", name=f"rtsb{c}")
                nc.scalar.activation(rt_sb[:], rt_ps[:], AF.Copy)
                for b_lo in range(2):
                    tails = mlp_batch(2 * c + b_lo, rt_sb, tails)
            for stage in tails:
                stage()
    return nc


def _get_nc():
    key = ('nc', _CACHE.get('cfg_nobias', False))
    if key not in _CACHE:
        _CACHE[key] = build_nc()
    return _CACHE[key]


def kernel(**inputs):
    import ml_dtypes

    from concourse.bass_utils import run_bass_kernel_spmd

    _install_wait_splitter()

    tf = np.asarray(inputs["temporal_features"], dtype=np.float32)
    ap = np.asarray(inputs["adj_param"], dtype=np.float32)
    W1 = np.asarray(inputs["W1"], dtype=np.float32)
    b1 = np.asarray(inputs["b1"], dtype=np.float32)
    W2 = np.asarray(inputs["W2"], dtype=np.float32)
    b2 = np.asarray(inputs["b2"], dtype=np.float32)
    W3 = np.asarray(inputs["W3"], dtype=np.float32)
    b3 = np.asarray(inputs["b3"], dtype=np.float32)

    bf = ml_dtypes.bfloat16
    Wi = np.ascontiguousarray((W1[:H] / T).astype(bf))
    Wj = np.ascontiguousarray((W1[H:] / T).astype(bf))
    b1c = b1.reshape(H, 1)
    b2p = np.concatenate([b2, b2]).reshape(H, 1)
    # W3 pair-block weights: chunk-pair d reads h2 of chunk 2d on PSUM
    # partitions 0:64 and chunk 2d+1 on 64:128; route each to F row 2d /
    # 2d+1 of the (8,512) accumulator (0.25 sym factor folded in).
    ND = NCH // 2
    W3p = np.zeros((H, ND, 8), np.float32)
    for d in range(ND):
        W3p[0 : H // 2, d, 2 * d] = 0.25 * W3[:, 0]
        W3p[H // 2 : H, d, 2 * d + 1] = 0.25 * W3[:, 0]
    W3p = np.ascontiguousarray(W3p.reshape(H, ND * 8).astype(bf))
    b3c = np.full((8, 1), 0.25 * float(b3[0]), np.float32)
    # xp mask: column (c, il, j) has ones at slots 8c+il and 64+j
    Smask = np.zeros((H, NCH, 8, N), np.float32)
    for c in range(NCH):
        for il in range(8):
            Smask[8 * c + il, c, il, :] = 1.0
    for j in range(N):
        Smask[64 + j, :, :, j] = 1.0
    Smask = np.ascontiguousarray(Smask.reshape(H, NCH * 8 * N).astype(bf))
    G = np.ascontiguousarray(0.25 * (ap + ap.T))
    I64np = np.eye(N, dtype=np.float32)
    I128np = np.eye(H, dtype=np.float32)

    shared = {
        "Wi": Wi, "Wj": Wj, "b1c": b1c,
        "W2": np.ascontiguousarray(W2.astype(bf)),
        "b2p": b2p, "W3p": W3p, "b3c": b3c, "Smask": Smask, "G": G,
        "I64": I64np, "I128": I128np,
    }
    in_maps = [
        {"tf": np.ascontiguousarray(tf[i * B_LOC : (i + 1) * B_LOC]), **shared}
        for i in range(NCORES)
    ]

    _CACHE['cfg_nobias'] = bool(
        not b1.any() and not b2.any() and not b3.any())
    nc = _get_nc()
    res = run_bass_kernel_spmd(nc, in_maps, core_ids=list(range(NCORES)),
                               **_CACHE.get('run_kwargs', {}))
    _CACHE['last_result'] = res
    out = np.concatenate([res.results[i]["out"] for i in range(NCORES)], axis=0)
    return np.ascontiguousarray(out.astype(np.float32))
